# revision 31
# baseline (speedup 1.0000x reference)
"""GATNet (2-layer GAT, PyG-style) forward on 8 Trainium2 NeuronCores.

Strategy (graph/data parallel, dst-sharded):
  - Nodes are partitioned into 8 contiguous shards of N/8; every edge is owned
    by the core owning its dst node (segment-softmax and scatter-add stay
    local to the dst shard, per the sharding hint).
  - Host-side prep (pure index/layout work on *input* data only): add self
    loops, sort edges by dst, bucket dst nodes by in-degree into groups of 128
    (partition rows) so that segment sums become free-dim row reductions with
    ~3% padding, and pre-gather x[src]/x[dst] per edge (the "halo gather" of
    input features, done at input-staging time).
  - Layer 1 never gathers 32-wide hidden features: h1 = x@W1 is linear in x,
    so per-edge work needs only x[src] (2 floats):
        e1 = leaky(x_src@v_s + x_dst@v_d),  v_s/v_d derived from W1, att_*.
        out1[d] = ((sum_e ee*x_src) @ W1) / sum_e ee
  - Layer 2 gathers a tiny per-node table t = [h2(2), a_s2(1), 0] (16B/row)
    with an indirect DMA from a table assembled across cores by an AllGather.
  - alpha outputs are computed in the bucketed layout and unscrambled on the
    host; exp() skips the segment-max shift (values are small; softmax is
    shift-invariant up to fp rounding).
"""

import math
import numpy as np

import concourse.bass as bass
import concourse.tile as tile
from concourse import mybir
from concourse.bass import IndirectOffsetOnAxis
from concourse.bass_utils import run_bass_kernel_spmd

# Collapse Tile's DMA-completion semaphore round-robin to a single lane per
# DGE type. With 8 lanes, an instruction depending on several in-flight DMAs
# (or a DMA reusing a slot) accumulates >2 distinct semaphore waits, which
# exceeds the per-instruction sync-wait budget in the walrus backend
# ("Too many sync wait commands"). One lane keeps every DMA wait a single
# max-tick semaphore compare; HWDGE DMAs complete in FIFO order per engine
# anyway, so the serialization this introduces is mostly nominal.
import concourse.tile_sem_assignment as _tsa
_tsa.NUM_HWDGE_SEMS = 1
_tsa.NUM_SWDGE_GLOBAL_SEMS = 1

F32 = mybir.dt.float32
I32 = mybir.dt.int32
AX = mybir.AxisListType
OP = mybir.AluOpType
ACT = mybir.ActivationFunctionType

NEG_SLOPE = 0.2
P = 128
SG_GROUPS = 8  # groups per supergroup (common padded degree within)

# weight layout inside the flat per-core weight vector
_OW1, _OAS1, _OAD1, _OB1, _OW2, _OAS2, _OAD2, _OB2, _WLEN = (
    0, 64, 96, 128, 160, 224, 226, 228, 230)


# --------------------------------------------------------------------------- #
# Host-side prep: sharding, degree bucketing, slot assignment.
# --------------------------------------------------------------------------- #
class _Plan:
    pass


def _prepare(x, edge_index, n_cores):
    pl = _Plan()
    N = x.shape[0]
    assert N % n_cores == 0
    NSH = N // n_cores
    G = (NSH + P - 1) // P
    NLOC = G * P
    assert NLOC > NSH, "need at least one dummy row per shard"

    src = np.concatenate([np.asarray(edge_index[0]), np.arange(N, dtype=np.int64)])
    dst = np.concatenate([np.asarray(edge_index[1]), np.arange(N, dtype=np.int64)])
    src = src.astype(np.int64)
    dst = dst.astype(np.int64)
    E = src.shape[0]
    deg = np.bincount(dst, minlength=N).astype(np.int64)

    # per-core degree-sorted node ranking
    rank_of = np.empty(N, dtype=np.int64)       # node -> local rank
    deg_sorted = np.zeros((n_cores, NLOC), dtype=np.int64)
    for c in range(n_cores):
        dc = deg[c * NSH:(c + 1) * NSH]
        order = np.argsort(-dc, kind="stable")
        rank_of[c * NSH + order] = np.arange(NSH)
        deg_sorted[c, :NSH] = dc[order]

    # fixed padded degree per supergroup = max over cores/groups inside
    n_sg = (G + SG_GROUPS - 1) // SG_GROUPS
    sg_sizes = [min(SG_GROUPS, G - s * SG_GROUPS) for s in range(n_sg)]
    sgD, col0 = [], np.zeros(G, dtype=np.int64)
    col = 0
    for s, ng in enumerate(sg_sizes):
        g0 = s * SG_GROUPS
        d = int(deg_sorted[:, g0 * P:(g0 + ng) * P].max())
        d = max(d, 1)
        sgD.append(d)
        for j in range(ng):
            col0[g0 + j] = col + j * d
        col += ng * d
    L = col

    # sg info tuples: (g0, ng, D, colstart)
    sginfo = []
    for s, ng in enumerate(sg_sizes):
        sginfo.append((s * SG_GROUPS, ng, sgD[s], int(col0[s * SG_GROUPS])))

    # global bucketed row per node (for the layer-2 gather table)
    core_of_node = np.arange(N) // NSH
    rk = rank_of
    globrow = core_of_node * NLOC + (rk // P) * P + (rk % P)
    # check: local pos = (g*128 + p) with g = rank//128, p = rank%128
    globrow = core_of_node * NLOC + (rk // P) * P + (rk % P)
    pad_row = NLOC - 1  # core 0's last dummy row

    # per-edge slot assignment
    core_of_edge = (dst // NSH).astype(np.int64)
    edge_slot = np.empty(E, dtype=np.int64)
    xs = x[src].astype(np.float32)

    pl.n_cores, pl.N, pl.NSH, pl.G, pl.NLOC, pl.L = n_cores, N, NSH, G, NLOC, L
    pl.sginfo, pl.pad_row = sginfo, pad_row
    pl.core_of_edge = core_of_edge
    pl.E = E

    xfeat = np.zeros((n_cores, 2, P, L), dtype=np.float32)
    xnode = np.zeros((n_cores, 2, P, G), dtype=np.float32)
    gidx = np.full((n_cores, P, L), pad_row, dtype=np.int32)
    npads = np.zeros((n_cores, P, G), dtype=np.float32)

    Darr = np.zeros(G, dtype=np.int64)
    for (g0, ng, D, _c0) in sginfo:
        Darr[g0:g0 + ng] = D

    for c in range(n_cores):
        e_ids = np.nonzero(core_of_edge == c)[0]
        r = rank_of[dst[e_ids]]
        o = np.argsort(r, kind="stable")
        e_s, r_s = e_ids[o], r[o]
        within = np.arange(len(r_s)) - np.searchsorted(r_s, r_s, side="left")
        g = r_s // P
        p = r_s % P
        colpos = col0[g] + within
        assert (within < Darr[g]).all()
        slot = p * L + colpos
        edge_slot[e_s] = slot
        xf = xfeat[c].reshape(2, P * L)
        xf[0][slot] = xs[e_s, 0]
        xf[1][slot] = xs[e_s, 1]
        gidx[c].reshape(P * L)[slot] = globrow[src[e_s]]
        npads[c] = (Darr[None, :] - deg_sorted[c].reshape(G, P).T).astype(np.float32)
        # x of the dst node sitting at (row p, group g)
        dc = deg[c * NSH:(c + 1) * NSH]
        order = np.argsort(-dc, kind="stable")       # rank -> local node id
        xn = x[c * NSH + order].astype(np.float32)   # [NSH, 2]
        rk_all = np.arange(NSH)
        xnode[c, 0, rk_all % P, rk_all // P] = xn[:, 0]
        xnode[c, 1, rk_all % P, rk_all // P] = xn[:, 1]

    pl.xnode = xnode

    pl.edge_slot = edge_slot
    pl.xfeat, pl.gidx, pl.npads = xfeat, gidx, npads
    pl.rank_of = rank_of
    return pl


def _pack_weights(W1, att_src1, att_dst1, b1, W2, att_src2, att_dst2, b2):
    w = np.zeros(_WLEN, dtype=np.float32)
    w[_OW1:_OW1 + 64] = W1.astype(np.float32).reshape(-1)        # [2,32] k-major
    w[_OAS1:_OAS1 + 32] = att_src1.astype(np.float32).reshape(-1)  # [4,8]
    w[_OAD1:_OAD1 + 32] = att_dst1.astype(np.float32).reshape(-1)
    w[_OB1:_OB1 + 32] = b1.astype(np.float32)
    w[_OW2:_OW2 + 64] = W2.astype(np.float32).reshape(-1)        # [32,2] c-major
    w[_OAS2:_OAS2 + 2] = att_src2.astype(np.float32).reshape(-1)
    w[_OAD2:_OAD2 + 2] = att_dst2.astype(np.float32).reshape(-1)
    w[_OB2:_OB2 + 2] = b2.astype(np.float32)
    return w.reshape(1, _WLEN)


def _spill_extra_waits(nc):
    """This walrus build accepts at most ONE embedded sync-wait per
    instruction ("Too many sync wait commands"). Move any extra waits onto
    NoOp instructions inserted right before the owner on the same engine —
    the sequencer blocks at the NoOp, which is semantically identical."""
    cnt = 0
    for fn in nc.m.functions:
        for bb in fn.blocks:
            out = []
            for inst in bb.instructions:
                si = inst.sync_info
                if si is not None and si.on_wait and len(si.on_wait) > 1:
                    waits = list(si.on_wait)
                    for w in waits[:-1]:
                        cnt += 1
                        out.append(mybir.InstNoOp(
                            name=f"wspill-{cnt}",
                            sync_info=mybir.SyncInfo(on_wait=[w], on_update=[]),
                            engine=inst.engine,
                            bass_nofuse=True,
                        ))
                    si.on_wait = [waits[-1]]
                out.append(inst)
            bb.instructions[:] = out


# --------------------------------------------------------------------------- #
# Device program.
# --------------------------------------------------------------------------- #
def _build(pl):
    n_cores, G, L, NLOC = pl.n_cores, pl.G, pl.L, pl.NLOC
    nc = bass.Bass(num_devices=n_cores)

    xfeat = nc.declare_dram_parameter("xfeat", [2, P, L], F32, isOutput=False)
    xnode = nc.declare_dram_parameter("xnode", [2, P, G], F32, isOutput=False)
    gidx = nc.declare_dram_parameter("gidx", [P, L], I32, isOutput=False)
    npads = nc.declare_dram_parameter("npads", [P, G], F32, isOutput=False)
    wflat = nc.declare_dram_parameter("wflat", [1, _WLEN], F32, isOutput=False)
    alpha1 = nc.declare_dram_parameter("alpha1", [P, 4, L], F32, isOutput=True)
    alpha2 = nc.declare_dram_parameter("alpha2", [P, L], F32, isOutput=True)
    out2 = nc.declare_dram_parameter("out2", [P, G, 2], F32, isOutput=True)

    t_loc = nc.dram_tensor("t_loc", [NLOC, 4], F32)
    t_glob = nc.dram_tensor("t_glob", [n_cores * NLOC, 4], F32, addr_space="Shared")

    with tile.TileContext(nc) as tc:
        with (
            tc.tile_pool(name="const", bufs=1) as cpool,
            tc.tile_pool(name="node", bufs=1) as npool,
            tc.tile_pool(name="big", bufs=1) as bpool,
            tc.tile_pool(name="work", bufs=2) as wpool,
            tc.tile_pool(name="dma", bufs=2) as dpool,
        ):
            # xs is only needed during the layer-1 edge stream; scope it in a
            # pool that closes before ee2 allocates so the two reuse the same
            # SBUF. (The gather-destination pool `dma` stays in fresh space:
            # DMA copies accept a single sync wait, so their first writes
            # cannot afford space-reuse hazards.)
            _xsp_cm = tc.tile_pool(name="xsp", bufs=1)
            xsp = _xsp_cm.__enter__()
            # ---------------- weights prep ---------------- #
            wb = cpool.tile([P, _WLEN], F32)
            nc.sync.dma_start(out=wb[:], in_=wflat[:].to_broadcast((P, _WLEN)))

            def w_ap(o, n=1):
                return wb[:, o:o + n]

            # v_s[k,h] = sum_c W1[k, 8h+c] * att_src1[h,c]  (and v_d)
            vs = cpool.tile([P, 8], F32)
            vd = cpool.tile([P, 8], F32)
            for out_t, att_o in ((vs, _OAS1), (vd, _OAD1)):
                prod = wpool.tile([P, 64], F32, tag="wprod")
                in0 = wb[:, _OW1:_OW1 + 64].rearrange(
                    "p (k h c) -> p k h c", k=2, h=4, c=8)
                in1 = wb[:, att_o:att_o + 32].rearrange(
                    "p (h c) -> p h c", h=4, c=8)[:, None, :, :].to_broadcast(
                    (P, 2, 4, 8))
                nc.vector.tensor_tensor(
                    out=prod[:].rearrange("p (k h c) -> p k h c", k=2, h=4, c=8),
                    in0=in0, in1=in1, op=OP.mult)
                nc.vector.tensor_reduce(
                    out=out_t[:],
                    in_=prod[:].rearrange("p (k h c) -> p k h c", k=2, h=4, c=8),
                    axis=AX.X, op=OP.add)

            # wts[c] = sum_j W2[c,j]*att_src2[j]; wtd likewise with att_dst2
            wts = cpool.tile([P, 32], F32)
            wtd = cpool.tile([P, 32], F32)
            w2_as_cj = wb[:, _OW2:_OW2 + 64].rearrange("p (c j) -> p c j", c=32, j=2)
            for out_t, att_o in ((wts, _OAS2), (wtd, _OAD2)):
                tmp = wpool.tile([P, 32], F32, tag="wtmp")
                nc.vector.tensor_scalar(
                    out=tmp[:], in0=w2_as_cj[:, :, 0], scalar1=w_ap(att_o),
                    scalar2=None, op0=OP.mult)
                nc.vector.scalar_tensor_tensor(
                    out=out_t[:], in0=w2_as_cj[:, :, 1], scalar=w_ap(att_o + 1),
                    in1=tmp[:], op0=OP.mult, op1=OP.add)

            # ub1 = elu(b1); asdum = as2-chain over ub1 (bitwise-matching chain)
            ub1 = cpool.tile([P, 32], F32)
            r32 = wpool.tile([P, 32], F32, tag="w32a")
            m32 = wpool.tile([P, 32], F32, tag="w32b")
            e32 = wpool.tile([P, 32], F32, tag="w32c")
            nc.scalar.activation(out=r32[:], in_=wb[:, _OB1:_OB1 + 32], func=ACT.Relu)
            nc.vector.tensor_scalar(
                out=m32[:], in0=wb[:, _OB1:_OB1 + 32], scalar1=0.0, scalar2=None,
                op0=OP.min)
            nc.scalar.activation(out=e32[:], in_=m32[:], func=ACT.Exp)
            nc.vector.scalar_tensor_tensor(
                out=ub1[:], in0=r32[:], scalar=-1.0, in1=e32[:],
                op0=OP.add, op1=OP.add)

            asdum = cpool.tile([P, 1], F32)
            acc_a = wpool.tile([P, 1], F32, tag="wacc_a")
            acc_b = wpool.tile([P, 1], F32, tag="wacc_b")
            accs = [acc_a, acc_b]
            nc.vector.tensor_scalar(
                out=accs[0][:], in0=ub1[:, 0:1], scalar1=wts[:, 0:1], scalar2=None,
                op0=OP.mult)
            for c in range(1, 32):
                dst_t = asdum if c == 31 else accs[c % 2]
                nc.vector.scalar_tensor_tensor(
                    out=dst_t[:], in0=ub1[:, c:c + 1], scalar=wts[:, c:c + 1],
                    in1=accs[(c - 1) % 2][:], op0=OP.mult, op1=OP.add)

            # ---------------- layer-1 edge stream ---------------- #
            ee = bpool.tile([P, 4, L], F32)
            gixall = bpool.tile([P, L], I32)
            nc.gpsimd.dma_start(out=gixall[:], in_=gidx[:])
            # tiny Pool-engine read of gixall: makes the Pool sequencer observe
            # the DMASW completion sem early, so the indirect gathers below
            # need no extra wait for it (DMA copies accept only 1 sync wait).
            obs = cpool.tile([1, 1], I32)
            nc.gpsimd.tensor_copy(out=obs[:], in_=gixall[0:1, 0:1])
            xs = xsp.tile([P, 2, L], F32)
            nc.sync.dma_start(
                out=xs[:], in_=xfeat[:].rearrange("f p l -> p f l"))
            s4 = npool.tile([P, 4, G], F32)
            xw0 = npool.tile([P, 4, G], F32)
            xw1 = npool.tile([P, 4, G], F32)
            npb = npool.tile([P, G], F32)
            nc.sync.dma_start(out=npb[:], in_=npads[:])
            xnb = npool.tile([P, 2, G], F32)
            nc.sync.dma_start(out=xnb[:], in_=xnode[:].rearrange("f p g -> p f g"))

            # a_d1 per dst node: adn1[p,h,g] = xnode@v_d
            adn1 = npool.tile([P, 4, G], F32)
            for h in range(4):
                tn = wpool.tile([P, G], F32, tag="tn")
                nc.vector.tensor_scalar(
                    out=tn[:], in0=xnb[:, 0, :], scalar1=vd[:, h:h + 1],
                    scalar2=None, op0=OP.mult)
                nc.vector.scalar_tensor_tensor(
                    out=adn1[:, h, :], in0=xnb[:, 1, :], scalar=vd[:, 4 + h:5 + h],
                    in1=tn[:], op0=OP.mult, op1=OP.add)
            # epad1 = exp(leaky(adn1)) — what a padded slot's ee evaluates to
            epad1 = npool.tile([P, 4, G], F32)
            lr1 = wpool.tile([P, 4, G], F32, tag="lr1")
            nc.vector.scalar_tensor_tensor(
                out=lr1[:], in0=adn1[:], scalar=NEG_SLOPE, in1=adn1[:],
                op0=OP.mult, op1=OP.max)
            nc.scalar.activation(out=epad1[:], in_=lr1[:], func=ACT.Exp)

            for (g0, ng, D, c0) in pl.sginfo:
                W = ng * D
                for h in range(4):
                    t0 = wpool.tile([P, W], F32, tag="t0")
                    t1 = wpool.tile([P, W], F32, tag="t1")
                    prh = wpool.tile([P, W], F32, tag="prh")
                    nc.vector.tensor_scalar(
                        out=t0[:], in0=xs[:, 0, c0:c0 + W], scalar1=vs[:, h:h + 1],
                        scalar2=None, op0=OP.mult)
                    nc.vector.scalar_tensor_tensor(
                        out=t1[:], in0=xs[:, 1, c0:c0 + W], scalar=vs[:, 4 + h:5 + h],
                        in1=t0[:], op0=OP.mult, op1=OP.add)
                    nc.vector.tensor_tensor(
                        out=t0[:].rearrange("p (g d) -> p g d", g=ng, d=D),
                        in0=t1[:].rearrange("p (g d) -> p g d", g=ng, d=D),
                        in1=adn1[:, h, g0:g0 + ng, None].to_broadcast((P, ng, D)),
                        op=OP.add)
                    # leaky relu: max(z, 0.2*z)
                    nc.vector.scalar_tensor_tensor(
                        out=prh[:], in0=t0[:], scalar=NEG_SLOPE, in1=t0[:],
                        op0=OP.mult, op1=OP.max)
                    nc.scalar.activation(
                        out=ee[:, h, c0:c0 + W], in_=prh[:], func=ACT.Exp)
                for h in range(4):
                    ee_g = ee[:, h, c0:c0 + W].rearrange("p (g d) -> p g d", g=ng, d=D)
                    nc.vector.tensor_reduce(
                        out=s4[:, h, g0:g0 + ng], in_=ee_g, axis=AX.X, op=OP.add)
                    for k, xw in ((0, xw0), (1, xw1)):
                        pr = wpool.tile([P, W], F32, tag="pr")
                        nc.vector.tensor_tensor(
                            out=pr[:], in0=ee[:, h, c0:c0 + W],
                            in1=xs[:, k, c0:c0 + W], op=OP.mult)
                        nc.vector.tensor_reduce(
                            out=xw[:, h, g0:g0 + ng],
                            in_=pr[:].rearrange("p (g d) -> p g d", g=ng, d=D),
                            axis=AX.X, op=OP.add)

            # ---------------- layer-1 node finalize ---------------- #
            _xsp_cm.__exit__(None, None, None)
            _e2_cm = tc.tile_pool(name="e2p", bufs=1)
            e2p = _e2_cm.__enter__()
            sinv1 = npool.tile([P, 4, G], F32)
            seps = wpool.tile([P, 4, G], F32, tag="seps")
            nc.vector.tensor_tensor(
                out=seps[:], in0=npb[:, None, :].to_broadcast((P, 4, G)),
                in1=epad1[:], op=OP.mult)
            nc.vector.tensor_tensor(
                out=seps[:], in0=s4[:], in1=seps[:], op=OP.subtract)
            nc.vector.tensor_scalar(
                out=seps[:], in0=seps[:], scalar1=1e-16, scalar2=None, op0=OP.add)
            nc.vector.reciprocal(out=sinv1[:], in_=seps[:])

            u = npool.tile([P, 32, G], F32)
            for hc in range(32):
                h = hc // 8
                ta = wpool.tile([P, G], F32, tag="ta")
                tb = wpool.tile([P, G], F32, tag="tb")
                nc.vector.tensor_scalar(
                    out=ta[:], in0=xw1[:, h, :], scalar1=w_ap(_OW1 + 32 + hc),
                    scalar2=None, op0=OP.mult)
                nc.vector.scalar_tensor_tensor(
                    out=tb[:], in0=xw0[:, h, :], scalar=w_ap(_OW1 + hc), in1=ta[:],
                    op0=OP.mult, op1=OP.add)
                nc.vector.tensor_tensor(
                    out=ta[:], in0=tb[:], in1=sinv1[:, h, :], op=OP.mult)
                rt = wpool.tile([P, G], F32, tag="rt")
                mt = wpool.tile([P, G], F32, tag="mt")
                et = wpool.tile([P, G], F32, tag="et")
                nc.scalar.activation(
                    out=rt[:], in_=ta[:], func=ACT.Relu, bias=w_ap(_OB1 + hc))
                nc.vector.tensor_scalar(
                    out=mt[:], in0=ta[:], scalar1=w_ap(_OB1 + hc), scalar2=0.0,
                    op0=OP.add, op1=OP.min)
                nc.scalar.activation(out=et[:], in_=mt[:], func=ACT.Exp)
                nc.vector.scalar_tensor_tensor(
                    out=u[:, hc, :], in0=rt[:], scalar=-1.0, in1=et[:],
                    op0=OP.add, op1=OP.add)

            # t-pack: [t0, t1, a_s2, 0] per node; a_d2 separately
            tpk = npool.tile([P, G, 4], F32)
            ad2n = npool.tile([P, G], F32)
            nc.vector.memset(tpk[:], 0.0)
            chains = (
                (lambda c: w_ap(_OW2 + 2 * c), tpk[:, :, 0]),
                (lambda c: w_ap(_OW2 + 2 * c + 1), tpk[:, :, 1]),
                (lambda c: wts[:, c:c + 1], tpk[:, :, 2]),
                (lambda c: wtd[:, c:c + 1], ad2n[:]),
            )
            for (scl, outslice) in chains:
                ca = wpool.tile([P, G], F32, tag="ca")
                cb = wpool.tile([P, G], F32, tag="cb")
                cc = [ca, cb]
                nc.vector.tensor_scalar(
                    out=cc[0][:], in0=u[:, 0, :], scalar1=scl(0), scalar2=None,
                    op0=OP.mult)
                for c in range(1, 32):
                    dst_t = outslice if c == 31 else cc[c % 2][:]
                    nc.vector.scalar_tensor_tensor(
                        out=dst_t, in0=u[:, c, :], scalar=scl(c),
                        in1=cc[(c - 1) % 2][:], op0=OP.mult, op1=OP.add)

            # epad = exp(leaky(a_d2 + asdum))
            epad = npool.tile([P, G], F32)
            pp = wpool.tile([P, G], F32, tag="pp")
            nc.vector.tensor_scalar(
                out=pp[:], in0=ad2n[:], scalar1=asdum[:, 0:1], scalar2=None,
                op0=OP.add)
            nc.vector.scalar_tensor_tensor(
                out=pp[:], in0=pp[:], scalar=NEG_SLOPE, in1=pp[:],
                op0=OP.mult, op1=OP.max)
            nc.scalar.activation(out=epad[:], in_=pp[:], func=ACT.Exp)

            # publish t-table, allgather
            nc.sync.dma_start(
                out=t_loc[:].rearrange("(g p) f -> p g f", p=P, g=G), in_=tpk[:])
            nc.gpsimd.collective_compute(
                "AllGather", OP.bypass,
                replica_groups=[list(range(pl.n_cores))],
                ins=[t_loc[:]], outs=[t_glob[:]])

            # ---------------- alpha1 ---------------- #
            for (g0, ng, D, c0) in pl.sginfo:
                W = ng * D
                for h in range(4):
                    a1 = wpool.tile([P, W], F32, tag="a1")
                    nc.vector.tensor_tensor(
                        out=a1[:].rearrange("p (g d) -> p g d", g=ng, d=D),
                        in0=ee[:, h, c0:c0 + W].rearrange(
                            "p (g d) -> p g d", g=ng, d=D),
                        in1=sinv1[:, h, g0:g0 + ng, None].to_broadcast((P, ng, D)),
                        op=OP.mult)
                    nc.sync.dma_start(out=alpha1[:, h, c0:c0 + W], in_=a1[:])

            # ---------------- layer-2 edge stream ---------------- #
            ee2 = e2p.tile([P, L], F32)
            s2 = npool.tile([P, G], F32)
            xw20 = npool.tile([P, G], F32)
            xw21 = npool.tile([P, G], F32)
            for (g0, ng, D, c0) in pl.sginfo:
                W = ng * D
                gt = dpool.tile([P, W, 4], F32, tag="gt")
                # the vector-indirect DMA consumes exactly one offset per
                # partition per instruction; gather column by column
                for j in range(W):
                    nc.gpsimd.indirect_dma_start(
                        out=gt[:, j, :], out_offset=None, in_=t_glob[:],
                        in_offset=IndirectOffsetOnAxis(
                            ap=gixall[:, c0 + j:c0 + j + 1], axis=0))
                p2 = wpool.tile([P, W], F32, tag="p2")
                nc.vector.tensor_tensor(
                    out=p2[:].rearrange("p (g d) -> p g d", g=ng, d=D),
                    in0=gt[:, :, 2].rearrange("p (g d) -> p g d", g=ng, d=D),
                    in1=ad2n[:, g0:g0 + ng, None].to_broadcast((P, ng, D)),
                    op=OP.add)
                nc.vector.scalar_tensor_tensor(
                    out=p2[:], in0=p2[:], scalar=NEG_SLOPE, in1=p2[:],
                    op0=OP.mult, op1=OP.max)
                nc.scalar.activation(out=ee2[:, c0:c0 + W], in_=p2[:], func=ACT.Exp)
                nc.vector.tensor_reduce(
                    out=s2[:, g0:g0 + ng],
                    in_=ee2[:, c0:c0 + W].rearrange("p (g d) -> p g d", g=ng, d=D),
                    axis=AX.X, op=OP.add)
                for k, xw in ((0, xw20), (1, xw21)):
                    pr2 = wpool.tile([P, W], F32, tag="pr2")
                    nc.vector.tensor_tensor(
                        out=pr2[:], in0=ee2[:, c0:c0 + W], in1=gt[:, :, k],
                        op=OP.mult)
                    nc.vector.tensor_reduce(
                        out=xw[:, g0:g0 + ng],
                        in_=pr2[:].rearrange("p (g d) -> p g d", g=ng, d=D),
                        axis=AX.X, op=OP.add)

            # ---------------- layer-2 finalize ---------------- #
            sinv2 = npool.tile([P, G], F32)
            t2a = wpool.tile([P, G], F32, tag="t2a")
            nc.vector.tensor_tensor(out=t2a[:], in0=npb[:], in1=epad[:], op=OP.mult)
            nc.vector.tensor_tensor(out=t2a[:], in0=s2[:], in1=t2a[:], op=OP.subtract)
            nc.vector.tensor_scalar(
                out=t2a[:], in0=t2a[:], scalar1=1e-16, scalar2=None, op0=OP.add)
            nc.vector.reciprocal(out=sinv2[:], in_=t2a[:])

            o2 = npool.tile([P, G, 2], F32)
            for j, xw in ((0, xw20), (1, xw21)):
                t2b = wpool.tile([P, G], F32, tag="t2b")
                nc.vector.tensor_tensor(out=t2b[:], in0=xw[:], in1=sinv2[:], op=OP.mult)
                nc.vector.tensor_scalar(
                    out=o2[:, :, j], in0=t2b[:], scalar1=w_ap(_OB2 + j),
                    scalar2=None, op0=OP.add)
            nc.sync.dma_start(out=out2[:], in_=o2[:])

            for (g0, ng, D, c0) in pl.sginfo:
                W = ng * D
                a2 = wpool.tile([P, W], F32, tag="a2")
                nc.vector.tensor_tensor(
                    out=a2[:].rearrange("p (g d) -> p g d", g=ng, d=D),
                    in0=ee2[:, c0:c0 + W].rearrange("p (g d) -> p g d", g=ng, d=D),
                    in1=sinv2[:, g0:g0 + ng, None].to_broadcast((P, ng, D)),
                    op=OP.mult)
                nc.sync.dma_start(out=alpha2[:, c0:c0 + W], in_=a2[:])

            _e2_cm.__exit__(None, None, None)

    _spill_extra_waits(nc)
    return nc


# --------------------------------------------------------------------------- #
# Entry points.
# --------------------------------------------------------------------------- #
def gat_forward(inputs, n_cores=8, trace=False, sim=False, tmpdir=None):
    x = np.asarray(inputs["x"], dtype=np.float32)
    edge_index = np.asarray(inputs["edge_index"])
    pl = _prepare(x, edge_index, n_cores)
    wfl = _pack_weights(
        np.asarray(inputs["W1"]), np.asarray(inputs["att_src1"]),
        np.asarray(inputs["att_dst1"]), np.asarray(inputs["b1"]),
        np.asarray(inputs["W2"]), np.asarray(inputs["att_src2"]),
        np.asarray(inputs["att_dst2"]), np.asarray(inputs["b2"]))
    nc = _build(pl)

    in_maps = [
        {"xfeat": pl.xfeat[c], "xnode": pl.xnode[c], "gidx": pl.gidx[c],
         "npads": pl.npads[c], "wflat": wfl}
        for c in range(n_cores)
    ]
    if sim:
        from concourse.bass_interp import MultiCoreSim
        ms = MultiCoreSim(nc, n_cores, num_workers=min(8, n_cores))
        for c in range(n_cores):
            for k, v in in_maps[c].items():
                ms.cores[c].tensor(k)[:] = v
        ms.simulate()
        results = [
            {k: np.array(ms.cores[c].tensor(k))
             for k in ("alpha1", "alpha2", "out2")}
            for c in range(n_cores)
        ]
        exec_ns = None
    else:
        r = run_bass_kernel_spmd(
            nc, in_maps, list(range(n_cores)), trace=trace, tmpdir=tmpdir)
        results = r.results
        exec_ns = r.exec_time_ns

    # unshard
    E, L, Pdim = pl.E, pl.L, P
    a1 = np.stack([results[c]["alpha1"] for c in range(n_cores)])  # [C,128,4,L]
    a2 = np.stack([results[c]["alpha2"] for c in range(n_cores)])  # [C,128,L]
    o2 = np.stack([results[c]["out2"] for c in range(n_cores)])    # [C,128,G,2]

    ce = pl.core_of_edge
    prow = pl.edge_slot // L
    pcol = pl.edge_slot % L
    alpha1_full = a1[ce, prow, :, pcol].astype(np.float32)         # [E,4]
    alpha2_full = a2[ce, prow, pcol][:, None].astype(np.float32)   # [E,1]
    rk = pl.rank_of
    cn = np.arange(pl.N) // pl.NSH
    out_full = o2[cn, rk % P, rk // P, :].astype(np.float32)       # [N,2]
    return (out_full, alpha1_full, alpha2_full), exec_ns


def kernel(**inputs):
    (out, a1, a2), _ = gat_forward(inputs, n_cores=8)
    return out, a1, a2


# revision 33
# speedup vs baseline: 1.0511x; 1.0511x over previous
"""GATNet (2-layer GAT, PyG-style) forward on 8 Trainium2 NeuronCores.

Strategy (graph/data parallel, dst-sharded):
  - Nodes are partitioned into 8 contiguous shards of N/8; every edge is owned
    by the core owning its dst node (segment-softmax and scatter-add stay
    local to the dst shard, per the sharding hint).
  - Host-side prep (pure index/layout work on *input* data only): add self
    loops, sort edges by dst, bucket dst nodes by in-degree into groups of 128
    (partition rows) so that segment sums become free-dim row reductions with
    ~3% padding, and pre-gather x[src]/x[dst] per edge (the "halo gather" of
    input features, done at input-staging time).
  - Layer 1 never gathers 32-wide hidden features: h1 = x@W1 is linear in x,
    so per-edge work needs only x[src] (2 floats):
        e1 = leaky(x_src@v_s + x_dst@v_d),  v_s/v_d derived from W1, att_*.
        out1[d] = ((sum_e ee*x_src) @ W1) / sum_e ee
  - Layer 2 gathers a tiny per-node table t = [h2(2), a_s2(1), 0] (16B/row)
    with an indirect DMA from a table assembled across cores by an AllGather.
  - alpha outputs are computed in the bucketed layout and unscrambled on the
    host; exp() skips the segment-max shift (values are small; softmax is
    shift-invariant up to fp rounding).
"""

import math
import numpy as np

import concourse.bass as bass
import concourse.tile as tile
from concourse import mybir
from concourse.bass import IndirectOffsetOnAxis
from concourse.bass_utils import run_bass_kernel_spmd

# Collapse Tile's DMA-completion semaphore round-robin to a single lane per
# DGE type. With 8 lanes, an instruction depending on several in-flight DMAs
# (or a DMA reusing a slot) accumulates >2 distinct semaphore waits, which
# exceeds the per-instruction sync-wait budget in the walrus backend
# ("Too many sync wait commands"). One lane keeps every DMA wait a single
# max-tick semaphore compare; HWDGE DMAs complete in FIFO order per engine
# anyway, so the serialization this introduces is mostly nominal.
import concourse.tile_sem_assignment as _tsa
_tsa.NUM_HWDGE_SEMS = 1
_tsa.NUM_SWDGE_GLOBAL_SEMS = 1

F32 = mybir.dt.float32
I32 = mybir.dt.int32
AX = mybir.AxisListType
OP = mybir.AluOpType
ACT = mybir.ActivationFunctionType

NEG_SLOPE = 0.2
P = 128
SG_GROUPS = 4  # groups per supergroup (common padded degree within)

# weight layout inside the flat per-core weight vector
_OW1, _OAS1, _OAD1, _OB1, _OW2, _OAS2, _OAD2, _OB2, _WLEN = (
    0, 64, 96, 128, 160, 224, 226, 228, 230)


# --------------------------------------------------------------------------- #
# Host-side prep: sharding, degree bucketing, slot assignment.
# --------------------------------------------------------------------------- #
class _Plan:
    pass


def _prepare(x, edge_index, n_cores):
    pl = _Plan()
    N = x.shape[0]
    assert N % n_cores == 0
    NSH = N // n_cores
    G = (NSH + P - 1) // P
    NLOC = G * P
    assert NLOC > NSH, "need at least one dummy row per shard"

    src = np.concatenate([np.asarray(edge_index[0]), np.arange(N, dtype=np.int64)])
    dst = np.concatenate([np.asarray(edge_index[1]), np.arange(N, dtype=np.int64)])
    src = src.astype(np.int64)
    dst = dst.astype(np.int64)
    E = src.shape[0]
    deg = np.bincount(dst, minlength=N).astype(np.int64)

    # per-core degree-sorted node ranking
    rank_of = np.empty(N, dtype=np.int64)       # node -> local rank
    deg_sorted = np.zeros((n_cores, NLOC), dtype=np.int64)
    for c in range(n_cores):
        dc = deg[c * NSH:(c + 1) * NSH]
        order = np.argsort(-dc, kind="stable")
        rank_of[c * NSH + order] = np.arange(NSH)
        deg_sorted[c, :NSH] = dc[order]

    # fixed padded degree per supergroup = max over cores/groups inside
    n_sg = (G + SG_GROUPS - 1) // SG_GROUPS
    sg_sizes = [min(SG_GROUPS, G - s * SG_GROUPS) for s in range(n_sg)]
    sgD, col0 = [], np.zeros(G, dtype=np.int64)
    col = 0
    for s, ng in enumerate(sg_sizes):
        g0 = s * SG_GROUPS
        d = int(deg_sorted[:, g0 * P:(g0 + ng) * P].max())
        d = max(d, 1)
        sgD.append(d)
        for j in range(ng):
            col0[g0 + j] = col + j * d
        col += ng * d
    L = col

    # sg info tuples: (g0, ng, D, colstart)
    sginfo = []
    for s, ng in enumerate(sg_sizes):
        sginfo.append((s * SG_GROUPS, ng, sgD[s], int(col0[s * SG_GROUPS])))

    # global bucketed row per node (for the layer-2 gather table)
    core_of_node = np.arange(N) // NSH
    rk = rank_of
    globrow = core_of_node * NLOC + (rk // P) * P + (rk % P)
    # check: local pos = (g*128 + p) with g = rank//128, p = rank%128
    globrow = core_of_node * NLOC + (rk // P) * P + (rk % P)
    pad_row = NLOC - 1  # core 0's last dummy row

    # per-edge slot assignment
    core_of_edge = (dst // NSH).astype(np.int64)
    edge_slot = np.empty(E, dtype=np.int64)
    xs = x[src].astype(np.float32)

    pl.n_cores, pl.N, pl.NSH, pl.G, pl.NLOC, pl.L = n_cores, N, NSH, G, NLOC, L
    pl.sginfo, pl.pad_row = sginfo, pad_row
    pl.core_of_edge = core_of_edge
    pl.E = E

    xfeat = np.zeros((n_cores, 2, P, L), dtype=np.float32)
    xnode = np.zeros((n_cores, 2, P, G), dtype=np.float32)
    gidx = np.full((n_cores, P, L), pad_row, dtype=np.int32)
    npads = np.zeros((n_cores, P, G), dtype=np.float32)

    Darr = np.zeros(G, dtype=np.int64)
    for (g0, ng, D, _c0) in sginfo:
        Darr[g0:g0 + ng] = D

    for c in range(n_cores):
        e_ids = np.nonzero(core_of_edge == c)[0]
        r = rank_of[dst[e_ids]]
        o = np.argsort(r, kind="stable")
        e_s, r_s = e_ids[o], r[o]
        within = np.arange(len(r_s)) - np.searchsorted(r_s, r_s, side="left")
        g = r_s // P
        p = r_s % P
        colpos = col0[g] + within
        assert (within < Darr[g]).all()
        slot = p * L + colpos
        edge_slot[e_s] = slot
        xf = xfeat[c].reshape(2, P * L)
        xf[0][slot] = xs[e_s, 0]
        xf[1][slot] = xs[e_s, 1]
        gidx[c].reshape(P * L)[slot] = globrow[src[e_s]]
        npads[c] = (Darr[None, :] - deg_sorted[c].reshape(G, P).T).astype(np.float32)
        # x of the dst node sitting at (row p, group g)
        dc = deg[c * NSH:(c + 1) * NSH]
        order = np.argsort(-dc, kind="stable")       # rank -> local node id
        xn = x[c * NSH + order].astype(np.float32)   # [NSH, 2]
        rk_all = np.arange(NSH)
        xnode[c, 0, rk_all % P, rk_all // P] = xn[:, 0]
        xnode[c, 1, rk_all % P, rk_all // P] = xn[:, 1]

    pl.xnode = xnode

    pl.edge_slot = edge_slot
    pl.xfeat, pl.gidx, pl.npads = xfeat, gidx, npads
    pl.rank_of = rank_of
    return pl


def _pack_weights(W1, att_src1, att_dst1, b1, W2, att_src2, att_dst2, b2):
    w = np.zeros(_WLEN, dtype=np.float32)
    w[_OW1:_OW1 + 64] = W1.astype(np.float32).reshape(-1)        # [2,32] k-major
    w[_OAS1:_OAS1 + 32] = att_src1.astype(np.float32).reshape(-1)  # [4,8]
    w[_OAD1:_OAD1 + 32] = att_dst1.astype(np.float32).reshape(-1)
    w[_OB1:_OB1 + 32] = b1.astype(np.float32)
    w[_OW2:_OW2 + 64] = W2.astype(np.float32).reshape(-1)        # [32,2] c-major
    w[_OAS2:_OAS2 + 2] = att_src2.astype(np.float32).reshape(-1)
    w[_OAD2:_OAD2 + 2] = att_dst2.astype(np.float32).reshape(-1)
    w[_OB2:_OB2 + 2] = b2.astype(np.float32)
    return w.reshape(1, _WLEN)


def _spill_extra_waits(nc):
    """This walrus build accepts at most ONE embedded sync-wait per
    instruction ("Too many sync wait commands"). Move any extra waits onto
    NoOp instructions inserted right before the owner on the same engine —
    the sequencer blocks at the NoOp, which is semantically identical."""
    cnt = 0
    for fn in nc.m.functions:
        for bb in fn.blocks:
            out = []
            for inst in bb.instructions:
                si = inst.sync_info
                if si is not None and si.on_wait and len(si.on_wait) > 1:
                    waits = list(si.on_wait)
                    for w in waits[:-1]:
                        cnt += 1
                        out.append(mybir.InstNoOp(
                            name=f"wspill-{cnt}",
                            sync_info=mybir.SyncInfo(on_wait=[w], on_update=[]),
                            engine=inst.engine,
                            bass_nofuse=True,
                        ))
                    si.on_wait = [waits[-1]]
                out.append(inst)
            bb.instructions[:] = out


# --------------------------------------------------------------------------- #
# Device program.
# --------------------------------------------------------------------------- #
def _build(pl):
    n_cores, G, L, NLOC = pl.n_cores, pl.G, pl.L, pl.NLOC
    nc = bass.Bass(num_devices=n_cores)

    xfeat = nc.declare_dram_parameter("xfeat", [2, P, L], F32, isOutput=False)
    xnode = nc.declare_dram_parameter("xnode", [2, P, G], F32, isOutput=False)
    gidx = nc.declare_dram_parameter("gidx", [P, L], I32, isOutput=False)
    npads = nc.declare_dram_parameter("npads", [P, G], F32, isOutput=False)
    wflat = nc.declare_dram_parameter("wflat", [1, _WLEN], F32, isOutput=False)
    alpha1 = nc.declare_dram_parameter("alpha1", [P, 4, L], F32, isOutput=True)
    alpha2 = nc.declare_dram_parameter("alpha2", [P, L], F32, isOutput=True)
    out2 = nc.declare_dram_parameter("out2", [P, G, 2], F32, isOutput=True)

    t_loc = nc.dram_tensor("t_loc", [NLOC, 4], F32)
    t_glob = nc.dram_tensor("t_glob", [n_cores * NLOC, 4], F32, addr_space="Shared")

    with tile.TileContext(nc) as tc:
        with (
            tc.tile_pool(name="const", bufs=1) as cpool,
            tc.tile_pool(name="node", bufs=1) as npool,
            tc.tile_pool(name="big", bufs=1) as bpool,
            tc.tile_pool(name="work", bufs=2) as wpool,
            tc.tile_pool(name="dma", bufs=3) as dpool,
        ):
            # xs is only needed during the layer-1 edge stream; scope it in a
            # pool that closes before ee2 allocates so the two reuse the same
            # SBUF. (The gather-destination pool `dma` stays in fresh space:
            # DMA copies accept a single sync wait, so their first writes
            # cannot afford space-reuse hazards.)
            _xsp_cm = tc.tile_pool(name="xsp", bufs=1)
            xsp = _xsp_cm.__enter__()
            # ---------------- weights prep ---------------- #
            wb = cpool.tile([P, _WLEN], F32)
            nc.sync.dma_start(out=wb[:], in_=wflat[:].to_broadcast((P, _WLEN)))

            def w_ap(o, n=1):
                return wb[:, o:o + n]

            # v_s[k,h] = sum_c W1[k, 8h+c] * att_src1[h,c]  (and v_d)
            vs = cpool.tile([P, 8], F32)
            vd = cpool.tile([P, 8], F32)
            for out_t, att_o in ((vs, _OAS1), (vd, _OAD1)):
                prod = wpool.tile([P, 64], F32, tag="wprod")
                in0 = wb[:, _OW1:_OW1 + 64].rearrange(
                    "p (k h c) -> p k h c", k=2, h=4, c=8)
                in1 = wb[:, att_o:att_o + 32].rearrange(
                    "p (h c) -> p h c", h=4, c=8)[:, None, :, :].to_broadcast(
                    (P, 2, 4, 8))
                nc.vector.tensor_tensor(
                    out=prod[:].rearrange("p (k h c) -> p k h c", k=2, h=4, c=8),
                    in0=in0, in1=in1, op=OP.mult)
                nc.vector.tensor_reduce(
                    out=out_t[:],
                    in_=prod[:].rearrange("p (k h c) -> p k h c", k=2, h=4, c=8),
                    axis=AX.X, op=OP.add)

            # wts[c] = sum_j W2[c,j]*att_src2[j]; wtd likewise with att_dst2
            wts = cpool.tile([P, 32], F32)
            wtd = cpool.tile([P, 32], F32)
            w2_as_cj = wb[:, _OW2:_OW2 + 64].rearrange("p (c j) -> p c j", c=32, j=2)
            for out_t, att_o in ((wts, _OAS2), (wtd, _OAD2)):
                tmp = wpool.tile([P, 32], F32, tag="wtmp")
                nc.vector.tensor_scalar(
                    out=tmp[:], in0=w2_as_cj[:, :, 0], scalar1=w_ap(att_o),
                    scalar2=None, op0=OP.mult)
                nc.vector.scalar_tensor_tensor(
                    out=out_t[:], in0=w2_as_cj[:, :, 1], scalar=w_ap(att_o + 1),
                    in1=tmp[:], op0=OP.mult, op1=OP.add)

            # ub1 = elu(b1); asdum = as2-chain over ub1 (bitwise-matching chain)
            ub1 = cpool.tile([P, 32], F32)
            r32 = wpool.tile([P, 32], F32, tag="w32a")
            m32 = wpool.tile([P, 32], F32, tag="w32b")
            e32 = wpool.tile([P, 32], F32, tag="w32c")
            nc.scalar.activation(out=r32[:], in_=wb[:, _OB1:_OB1 + 32], func=ACT.Relu)
            nc.vector.tensor_scalar(
                out=m32[:], in0=wb[:, _OB1:_OB1 + 32], scalar1=0.0, scalar2=None,
                op0=OP.min)
            nc.scalar.activation(out=e32[:], in_=m32[:], func=ACT.Exp)
            nc.vector.scalar_tensor_tensor(
                out=ub1[:], in0=r32[:], scalar=-1.0, in1=e32[:],
                op0=OP.add, op1=OP.add)

            asdum = cpool.tile([P, 1], F32)
            acc_a = wpool.tile([P, 1], F32, tag="wacc_a")
            acc_b = wpool.tile([P, 1], F32, tag="wacc_b")
            accs = [acc_a, acc_b]
            nc.vector.tensor_scalar(
                out=accs[0][:], in0=ub1[:, 0:1], scalar1=wts[:, 0:1], scalar2=None,
                op0=OP.mult)
            for c in range(1, 32):
                dst_t = asdum if c == 31 else accs[c % 2]
                nc.vector.scalar_tensor_tensor(
                    out=dst_t[:], in0=ub1[:, c:c + 1], scalar=wts[:, c:c + 1],
                    in1=accs[(c - 1) % 2][:], op0=OP.mult, op1=OP.add)

            # ---------------- layer-1 edge stream ---------------- #
            ee = bpool.tile([P, 4, L], F32)
            gixall = bpool.tile([P, L], I32)
            nc.gpsimd.dma_start(out=gixall[:], in_=gidx[:])
            # tiny Pool-engine read of gixall: makes the Pool sequencer observe
            # the DMASW completion sem early, so the indirect gathers below
            # need no extra wait for it (DMA copies accept only 1 sync wait).
            obs = cpool.tile([1, 1], I32)
            nc.gpsimd.tensor_copy(out=obs[:], in_=gixall[0:1, 0:1])
            xs = xsp.tile([P, 2, L], F32)
            nc.sync.dma_start(
                out=xs[:], in_=xfeat[:].rearrange("f p l -> p f l"))
            s4 = npool.tile([P, 4, G], F32)
            xw0 = npool.tile([P, 4, G], F32)
            xw1 = npool.tile([P, 4, G], F32)
            npb = npool.tile([P, G], F32)
            nc.sync.dma_start(out=npb[:], in_=npads[:])
            xnb = npool.tile([P, 2, G], F32)
            nc.sync.dma_start(out=xnb[:], in_=xnode[:].rearrange("f p g -> p f g"))

            # a_d1 per dst node: adn1[p,h,g] = xnode@v_d
            adn1 = npool.tile([P, 4, G], F32)
            for h in range(4):
                tn = wpool.tile([P, G], F32, tag="tn")
                nc.vector.tensor_scalar(
                    out=tn[:], in0=xnb[:, 0, :], scalar1=vd[:, h:h + 1],
                    scalar2=None, op0=OP.mult)
                nc.vector.scalar_tensor_tensor(
                    out=adn1[:, h, :], in0=xnb[:, 1, :], scalar=vd[:, 4 + h:5 + h],
                    in1=tn[:], op0=OP.mult, op1=OP.add)
            # epad1 = exp(leaky(adn1)) — what a padded slot's ee evaluates to
            epad1 = npool.tile([P, 4, G], F32)
            lr1 = wpool.tile([P, 4, G], F32, tag="lr1")
            nc.vector.scalar_tensor_tensor(
                out=lr1[:], in0=adn1[:], scalar=NEG_SLOPE, in1=adn1[:],
                op0=OP.mult, op1=OP.max)
            nc.scalar.activation(out=epad1[:], in_=lr1[:], func=ACT.Exp)

            for (g0, ng, D, c0) in pl.sginfo:
                W = ng * D
                for h in range(4):
                    t0 = wpool.tile([P, W], F32, tag="t0")
                    t1 = wpool.tile([P, W], F32, tag="t1")
                    prh = wpool.tile([P, W], F32, tag="prh")
                    nc.vector.tensor_scalar(
                        out=t0[:], in0=xs[:, 0, c0:c0 + W], scalar1=vs[:, h:h + 1],
                        scalar2=None, op0=OP.mult)
                    nc.vector.scalar_tensor_tensor(
                        out=t1[:], in0=xs[:, 1, c0:c0 + W], scalar=vs[:, 4 + h:5 + h],
                        in1=t0[:], op0=OP.mult, op1=OP.add)
                    nc.vector.tensor_tensor(
                        out=t0[:].rearrange("p (g d) -> p g d", g=ng, d=D),
                        in0=t1[:].rearrange("p (g d) -> p g d", g=ng, d=D),
                        in1=adn1[:, h, g0:g0 + ng, None].to_broadcast((P, ng, D)),
                        op=OP.add)
                    # leaky relu: max(z, 0.2*z)
                    nc.vector.scalar_tensor_tensor(
                        out=prh[:], in0=t0[:], scalar=NEG_SLOPE, in1=t0[:],
                        op0=OP.mult, op1=OP.max)
                    nc.scalar.activation(
                        out=ee[:, h, c0:c0 + W], in_=prh[:], func=ACT.Exp)
                for h in range(4):
                    ee_g = ee[:, h, c0:c0 + W].rearrange("p (g d) -> p g d", g=ng, d=D)
                    nc.vector.tensor_reduce(
                        out=s4[:, h, g0:g0 + ng], in_=ee_g, axis=AX.X, op=OP.add)
                    for k, xw in ((0, xw0), (1, xw1)):
                        pr = wpool.tile([P, W], F32, tag="pr")
                        nc.vector.tensor_tensor(
                            out=pr[:], in0=ee[:, h, c0:c0 + W],
                            in1=xs[:, k, c0:c0 + W], op=OP.mult)
                        nc.vector.tensor_reduce(
                            out=xw[:, h, g0:g0 + ng],
                            in_=pr[:].rearrange("p (g d) -> p g d", g=ng, d=D),
                            axis=AX.X, op=OP.add)

            # ---------------- layer-1 node finalize ---------------- #
            _xsp_cm.__exit__(None, None, None)
            _e2_cm = tc.tile_pool(name="e2p", bufs=1)
            e2p = _e2_cm.__enter__()
            sinv1 = npool.tile([P, 4, G], F32)
            seps = wpool.tile([P, 4, G], F32, tag="seps")
            nc.vector.tensor_tensor(
                out=seps[:], in0=npb[:, None, :].to_broadcast((P, 4, G)),
                in1=epad1[:], op=OP.mult)
            nc.vector.tensor_tensor(
                out=seps[:], in0=s4[:], in1=seps[:], op=OP.subtract)
            nc.vector.tensor_scalar(
                out=seps[:], in0=seps[:], scalar1=1e-16, scalar2=None, op0=OP.add)
            nc.vector.reciprocal(out=sinv1[:], in_=seps[:])

            u = npool.tile([P, 32, G], F32)
            for hc in range(32):
                h = hc // 8
                ta = wpool.tile([P, G], F32, tag="ta")
                tb = wpool.tile([P, G], F32, tag="tb")
                nc.vector.tensor_scalar(
                    out=ta[:], in0=xw1[:, h, :], scalar1=w_ap(_OW1 + 32 + hc),
                    scalar2=None, op0=OP.mult)
                nc.vector.scalar_tensor_tensor(
                    out=tb[:], in0=xw0[:, h, :], scalar=w_ap(_OW1 + hc), in1=ta[:],
                    op0=OP.mult, op1=OP.add)
                nc.vector.tensor_tensor(
                    out=ta[:], in0=tb[:], in1=sinv1[:, h, :], op=OP.mult)
                rt = wpool.tile([P, G], F32, tag="rt")
                mt = wpool.tile([P, G], F32, tag="mt")
                et = wpool.tile([P, G], F32, tag="et")
                nc.scalar.activation(
                    out=rt[:], in_=ta[:], func=ACT.Relu, bias=w_ap(_OB1 + hc))
                nc.vector.tensor_scalar(
                    out=mt[:], in0=ta[:], scalar1=w_ap(_OB1 + hc), scalar2=0.0,
                    op0=OP.add, op1=OP.min)
                nc.scalar.activation(out=et[:], in_=mt[:], func=ACT.Exp)
                nc.vector.scalar_tensor_tensor(
                    out=u[:, hc, :], in0=rt[:], scalar=-1.0, in1=et[:],
                    op0=OP.add, op1=OP.add)

            # t-pack: [t0, t1, a_s2, 0] per node; a_d2 separately
            tpk = npool.tile([P, G, 4], F32)
            ad2n = npool.tile([P, G], F32)
            nc.vector.memset(tpk[:], 0.0)
            chains = (
                (lambda c: w_ap(_OW2 + 2 * c), tpk[:, :, 0]),
                (lambda c: w_ap(_OW2 + 2 * c + 1), tpk[:, :, 1]),
                (lambda c: wts[:, c:c + 1], tpk[:, :, 2]),
                (lambda c: wtd[:, c:c + 1], ad2n[:]),
            )
            for (scl, outslice) in chains:
                ca = wpool.tile([P, G], F32, tag="ca")
                cb = wpool.tile([P, G], F32, tag="cb")
                cc = [ca, cb]
                nc.vector.tensor_scalar(
                    out=cc[0][:], in0=u[:, 0, :], scalar1=scl(0), scalar2=None,
                    op0=OP.mult)
                for c in range(1, 32):
                    dst_t = outslice if c == 31 else cc[c % 2][:]
                    nc.vector.scalar_tensor_tensor(
                        out=dst_t, in0=u[:, c, :], scalar=scl(c),
                        in1=cc[(c - 1) % 2][:], op0=OP.mult, op1=OP.add)

            # epad = exp(leaky(a_d2 + asdum))
            epad = npool.tile([P, G], F32)
            pp = wpool.tile([P, G], F32, tag="pp")
            nc.vector.tensor_scalar(
                out=pp[:], in0=ad2n[:], scalar1=asdum[:, 0:1], scalar2=None,
                op0=OP.add)
            nc.vector.scalar_tensor_tensor(
                out=pp[:], in0=pp[:], scalar=NEG_SLOPE, in1=pp[:],
                op0=OP.mult, op1=OP.max)
            nc.scalar.activation(out=epad[:], in_=pp[:], func=ACT.Exp)

            # publish t-table, allgather
            nc.sync.dma_start(
                out=t_loc[:].rearrange("(g p) f -> p g f", p=P, g=G), in_=tpk[:])
            nc.gpsimd.collective_compute(
                "AllGather", OP.bypass,
                replica_groups=[list(range(pl.n_cores))],
                ins=[t_loc[:]], outs=[t_glob[:]])

            # ---------------- alpha1 ---------------- #
            for (g0, ng, D, c0) in pl.sginfo:
                W = ng * D
                for h in range(4):
                    a1 = wpool.tile([P, W], F32, tag="a1")
                    nc.vector.tensor_tensor(
                        out=a1[:].rearrange("p (g d) -> p g d", g=ng, d=D),
                        in0=ee[:, h, c0:c0 + W].rearrange(
                            "p (g d) -> p g d", g=ng, d=D),
                        in1=sinv1[:, h, g0:g0 + ng, None].to_broadcast((P, ng, D)),
                        op=OP.mult)
                    nc.sync.dma_start(out=alpha1[:, h, c0:c0 + W], in_=a1[:])

            # ---------------- layer-2 edge stream ---------------- #
            ee2 = e2p.tile([P, L], F32)
            s2 = npool.tile([P, G], F32)
            xw20 = npool.tile([P, G], F32)
            xw21 = npool.tile([P, G], F32)
            for (g0, ng, D, c0) in pl.sginfo:
                W = ng * D
                gt = dpool.tile([P, W, 4], F32, tag="gt")
                # the vector-indirect DMA consumes exactly one offset per
                # partition per instruction; gather column by column
                for j in range(W):
                    nc.gpsimd.indirect_dma_start(
                        out=gt[:, j, :], out_offset=None, in_=t_glob[:],
                        in_offset=IndirectOffsetOnAxis(
                            ap=gixall[:, c0 + j:c0 + j + 1], axis=0))
                p2 = wpool.tile([P, W], F32, tag="p2")
                nc.vector.tensor_tensor(
                    out=p2[:].rearrange("p (g d) -> p g d", g=ng, d=D),
                    in0=gt[:, :, 2].rearrange("p (g d) -> p g d", g=ng, d=D),
                    in1=ad2n[:, g0:g0 + ng, None].to_broadcast((P, ng, D)),
                    op=OP.add)
                nc.vector.scalar_tensor_tensor(
                    out=p2[:], in0=p2[:], scalar=NEG_SLOPE, in1=p2[:],
                    op0=OP.mult, op1=OP.max)
                nc.scalar.activation(out=ee2[:, c0:c0 + W], in_=p2[:], func=ACT.Exp)
                nc.vector.tensor_reduce(
                    out=s2[:, g0:g0 + ng],
                    in_=ee2[:, c0:c0 + W].rearrange("p (g d) -> p g d", g=ng, d=D),
                    axis=AX.X, op=OP.add)
                for k, xw in ((0, xw20), (1, xw21)):
                    pr2 = wpool.tile([P, W], F32, tag="pr2")
                    nc.vector.tensor_tensor(
                        out=pr2[:], in0=ee2[:, c0:c0 + W], in1=gt[:, :, k],
                        op=OP.mult)
                    nc.vector.tensor_reduce(
                        out=xw[:, g0:g0 + ng],
                        in_=pr2[:].rearrange("p (g d) -> p g d", g=ng, d=D),
                        axis=AX.X, op=OP.add)

            # ---------------- layer-2 finalize ---------------- #
            sinv2 = npool.tile([P, G], F32)
            t2a = wpool.tile([P, G], F32, tag="t2a")
            nc.vector.tensor_tensor(out=t2a[:], in0=npb[:], in1=epad[:], op=OP.mult)
            nc.vector.tensor_tensor(out=t2a[:], in0=s2[:], in1=t2a[:], op=OP.subtract)
            nc.vector.tensor_scalar(
                out=t2a[:], in0=t2a[:], scalar1=1e-16, scalar2=None, op0=OP.add)
            nc.vector.reciprocal(out=sinv2[:], in_=t2a[:])

            o2 = npool.tile([P, G, 2], F32)
            for j, xw in ((0, xw20), (1, xw21)):
                t2b = wpool.tile([P, G], F32, tag="t2b")
                nc.vector.tensor_tensor(out=t2b[:], in0=xw[:], in1=sinv2[:], op=OP.mult)
                nc.vector.tensor_scalar(
                    out=o2[:, :, j], in0=t2b[:], scalar1=w_ap(_OB2 + j),
                    scalar2=None, op0=OP.add)
            nc.sync.dma_start(out=out2[:], in_=o2[:])

            for (g0, ng, D, c0) in pl.sginfo:
                W = ng * D
                a2 = wpool.tile([P, W], F32, tag="a2")
                nc.vector.tensor_tensor(
                    out=a2[:].rearrange("p (g d) -> p g d", g=ng, d=D),
                    in0=ee2[:, c0:c0 + W].rearrange("p (g d) -> p g d", g=ng, d=D),
                    in1=sinv2[:, g0:g0 + ng, None].to_broadcast((P, ng, D)),
                    op=OP.mult)
                nc.sync.dma_start(out=alpha2[:, c0:c0 + W], in_=a2[:])

            _e2_cm.__exit__(None, None, None)

    _spill_extra_waits(nc)
    return nc


# --------------------------------------------------------------------------- #
# Entry points.
# --------------------------------------------------------------------------- #
def gat_forward(inputs, n_cores=8, trace=False, sim=False, tmpdir=None):
    x = np.asarray(inputs["x"], dtype=np.float32)
    edge_index = np.asarray(inputs["edge_index"])
    pl = _prepare(x, edge_index, n_cores)
    wfl = _pack_weights(
        np.asarray(inputs["W1"]), np.asarray(inputs["att_src1"]),
        np.asarray(inputs["att_dst1"]), np.asarray(inputs["b1"]),
        np.asarray(inputs["W2"]), np.asarray(inputs["att_src2"]),
        np.asarray(inputs["att_dst2"]), np.asarray(inputs["b2"]))
    nc = _build(pl)

    in_maps = [
        {"xfeat": pl.xfeat[c], "xnode": pl.xnode[c], "gidx": pl.gidx[c],
         "npads": pl.npads[c], "wflat": wfl}
        for c in range(n_cores)
    ]
    if sim:
        from concourse.bass_interp import MultiCoreSim
        ms = MultiCoreSim(nc, n_cores, num_workers=min(8, n_cores))
        for c in range(n_cores):
            for k, v in in_maps[c].items():
                ms.cores[c].tensor(k)[:] = v
        ms.simulate()
        results = [
            {k: np.array(ms.cores[c].tensor(k))
             for k in ("alpha1", "alpha2", "out2")}
            for c in range(n_cores)
        ]
        exec_ns = None
    else:
        r = run_bass_kernel_spmd(
            nc, in_maps, list(range(n_cores)), trace=trace, tmpdir=tmpdir)
        results = r.results
        exec_ns = r.exec_time_ns

    # unshard
    E, L, Pdim = pl.E, pl.L, P
    a1 = np.stack([results[c]["alpha1"] for c in range(n_cores)])  # [C,128,4,L]
    a2 = np.stack([results[c]["alpha2"] for c in range(n_cores)])  # [C,128,L]
    o2 = np.stack([results[c]["out2"] for c in range(n_cores)])    # [C,128,G,2]

    ce = pl.core_of_edge
    prow = pl.edge_slot // L
    pcol = pl.edge_slot % L
    alpha1_full = a1[ce, prow, :, pcol].astype(np.float32)         # [E,4]
    alpha2_full = a2[ce, prow, pcol][:, None].astype(np.float32)   # [E,1]
    rk = pl.rank_of
    cn = np.arange(pl.N) // pl.NSH
    out_full = o2[cn, rk % P, rk // P, :].astype(np.float32)       # [N,2]
    return (out_full, alpha1_full, alpha2_full), exec_ns


def kernel(**inputs):
    (out, a1, a2), _ = gat_forward(inputs, n_cores=8)
    return out, a1, a2


# revision 34
# speedup vs baseline: 1.6980x; 1.6154x over previous
"""GATNet (2-layer GAT, PyG-style) forward on 8 Trainium2 NeuronCores.

Strategy (graph/data parallel, dst-sharded):
  - Nodes are partitioned into 8 contiguous shards of N/8; every edge is owned
    by the core owning its dst node (segment-softmax and scatter-add stay
    local to the dst shard, per the sharding hint).
  - Host-side prep (pure index/layout work on *input* data only): add self
    loops, sort edges by dst, bucket dst nodes by in-degree into groups of 128
    (partition rows) so that segment sums become free-dim row reductions with
    ~3% padding, and pre-gather x[src]/x[dst] per edge (the "halo gather" of
    input features, done at input-staging time).
  - Layer 1 never gathers 32-wide hidden features: h1 = x@W1 is linear in x,
    so per-edge work needs only x[src] (2 floats):
        e1 = leaky(x_src@v_s + x_dst@v_d),  v_s/v_d derived from W1, att_*.
        out1[d] = ((sum_e ee*x_src) @ W1) / sum_e ee
  - Layer 2 gathers a tiny per-node table t = [h2(2), a_s2(1), 0] (16B/row)
    with an indirect DMA from a table assembled across cores by an AllGather.
  - alpha outputs are computed in the bucketed layout and unscrambled on the
    host; exp() skips the segment-max shift (values are small; softmax is
    shift-invariant up to fp rounding).
"""

import math
import numpy as np

import concourse.bass as bass
import concourse.tile as tile
from concourse import mybir
from concourse.bass import IndirectOffsetOnAxis
from concourse.bass_utils import run_bass_kernel_spmd

# Collapse Tile's DMA-completion semaphore round-robin to a single lane per
# DGE type. With 8 lanes, an instruction depending on several in-flight DMAs
# (or a DMA reusing a slot) accumulates >2 distinct semaphore waits, which
# exceeds the per-instruction sync-wait budget in the walrus backend
# ("Too many sync wait commands"). One lane keeps every DMA wait a single
# max-tick semaphore compare; HWDGE DMAs complete in FIFO order per engine
# anyway, so the serialization this introduces is mostly nominal.
import concourse.tile_sem_assignment as _tsa
_tsa.NUM_HWDGE_SEMS = 1
# SWDGE keeps all 8 completion lanes: the per-column indirect gathers are
# issued on the SWDGE queue, and with a single lane Tile chains each gather
# on the previous one's completion (~2.7us serialized cadence). Eight lanes
# keep 8 gathers in flight; any instruction that accumulates >1 sync wait as
# a result is handled by _spill_extra_waits.
_tsa.NUM_SWDGE_GLOBAL_SEMS = 8

F32 = mybir.dt.float32
I32 = mybir.dt.int32
AX = mybir.AxisListType
OP = mybir.AluOpType
ACT = mybir.ActivationFunctionType

NEG_SLOPE = 0.2
P = 128
SG_GROUPS = 4  # groups per supergroup (common padded degree within)

# weight layout inside the flat per-core weight vector
_OW1, _OAS1, _OAD1, _OB1, _OW2, _OAS2, _OAD2, _OB2, _WLEN = (
    0, 64, 96, 128, 160, 224, 226, 228, 230)


# --------------------------------------------------------------------------- #
# Host-side prep: sharding, degree bucketing, slot assignment.
# --------------------------------------------------------------------------- #
class _Plan:
    pass


def _prepare(x, edge_index, n_cores):
    pl = _Plan()
    N = x.shape[0]
    assert N % n_cores == 0
    NSH = N // n_cores
    G = (NSH + P - 1) // P
    NLOC = G * P
    assert NLOC > NSH, "need at least one dummy row per shard"

    src = np.concatenate([np.asarray(edge_index[0]), np.arange(N, dtype=np.int64)])
    dst = np.concatenate([np.asarray(edge_index[1]), np.arange(N, dtype=np.int64)])
    src = src.astype(np.int64)
    dst = dst.astype(np.int64)
    E = src.shape[0]
    deg = np.bincount(dst, minlength=N).astype(np.int64)

    # per-core degree-sorted node ranking
    rank_of = np.empty(N, dtype=np.int64)       # node -> local rank
    deg_sorted = np.zeros((n_cores, NLOC), dtype=np.int64)
    for c in range(n_cores):
        dc = deg[c * NSH:(c + 1) * NSH]
        order = np.argsort(-dc, kind="stable")
        rank_of[c * NSH + order] = np.arange(NSH)
        deg_sorted[c, :NSH] = dc[order]

    # fixed padded degree per supergroup = max over cores/groups inside
    n_sg = (G + SG_GROUPS - 1) // SG_GROUPS
    sg_sizes = [min(SG_GROUPS, G - s * SG_GROUPS) for s in range(n_sg)]
    sgD, col0 = [], np.zeros(G, dtype=np.int64)
    col = 0
    for s, ng in enumerate(sg_sizes):
        g0 = s * SG_GROUPS
        d = int(deg_sorted[:, g0 * P:(g0 + ng) * P].max())
        d = max(d, 1)
        sgD.append(d)
        for j in range(ng):
            col0[g0 + j] = col + j * d
        col += ng * d
    L = col

    # sg info tuples: (g0, ng, D, colstart)
    sginfo = []
    for s, ng in enumerate(sg_sizes):
        sginfo.append((s * SG_GROUPS, ng, sgD[s], int(col0[s * SG_GROUPS])))

    # global bucketed row per node (for the layer-2 gather table)
    core_of_node = np.arange(N) // NSH
    rk = rank_of
    globrow = core_of_node * NLOC + (rk // P) * P + (rk % P)
    # check: local pos = (g*128 + p) with g = rank//128, p = rank%128
    globrow = core_of_node * NLOC + (rk // P) * P + (rk % P)
    pad_row = NLOC - 1  # core 0's last dummy row

    # per-edge slot assignment
    core_of_edge = (dst // NSH).astype(np.int64)
    edge_slot = np.empty(E, dtype=np.int64)
    xs = x[src].astype(np.float32)

    pl.n_cores, pl.N, pl.NSH, pl.G, pl.NLOC, pl.L = n_cores, N, NSH, G, NLOC, L
    pl.sginfo, pl.pad_row = sginfo, pad_row
    pl.core_of_edge = core_of_edge
    pl.E = E

    xfeat = np.zeros((n_cores, 2, P, L), dtype=np.float32)
    xnode = np.zeros((n_cores, 2, P, G), dtype=np.float32)
    gidx = np.full((n_cores, P, L), pad_row, dtype=np.int32)
    npads = np.zeros((n_cores, P, G), dtype=np.float32)

    Darr = np.zeros(G, dtype=np.int64)
    for (g0, ng, D, _c0) in sginfo:
        Darr[g0:g0 + ng] = D

    for c in range(n_cores):
        e_ids = np.nonzero(core_of_edge == c)[0]
        r = rank_of[dst[e_ids]]
        o = np.argsort(r, kind="stable")
        e_s, r_s = e_ids[o], r[o]
        within = np.arange(len(r_s)) - np.searchsorted(r_s, r_s, side="left")
        g = r_s // P
        p = r_s % P
        colpos = col0[g] + within
        assert (within < Darr[g]).all()
        slot = p * L + colpos
        edge_slot[e_s] = slot
        xf = xfeat[c].reshape(2, P * L)
        xf[0][slot] = xs[e_s, 0]
        xf[1][slot] = xs[e_s, 1]
        gidx[c].reshape(P * L)[slot] = globrow[src[e_s]]
        npads[c] = (Darr[None, :] - deg_sorted[c].reshape(G, P).T).astype(np.float32)
        # x of the dst node sitting at (row p, group g)
        dc = deg[c * NSH:(c + 1) * NSH]
        order = np.argsort(-dc, kind="stable")       # rank -> local node id
        xn = x[c * NSH + order].astype(np.float32)   # [NSH, 2]
        rk_all = np.arange(NSH)
        xnode[c, 0, rk_all % P, rk_all // P] = xn[:, 0]
        xnode[c, 1, rk_all % P, rk_all // P] = xn[:, 1]

    pl.xnode = xnode

    pl.edge_slot = edge_slot
    pl.xfeat, pl.gidx, pl.npads = xfeat, gidx, npads
    pl.rank_of = rank_of
    return pl


def _pack_weights(W1, att_src1, att_dst1, b1, W2, att_src2, att_dst2, b2):
    w = np.zeros(_WLEN, dtype=np.float32)
    w[_OW1:_OW1 + 64] = W1.astype(np.float32).reshape(-1)        # [2,32] k-major
    w[_OAS1:_OAS1 + 32] = att_src1.astype(np.float32).reshape(-1)  # [4,8]
    w[_OAD1:_OAD1 + 32] = att_dst1.astype(np.float32).reshape(-1)
    w[_OB1:_OB1 + 32] = b1.astype(np.float32)
    w[_OW2:_OW2 + 64] = W2.astype(np.float32).reshape(-1)        # [32,2] c-major
    w[_OAS2:_OAS2 + 2] = att_src2.astype(np.float32).reshape(-1)
    w[_OAD2:_OAD2 + 2] = att_dst2.astype(np.float32).reshape(-1)
    w[_OB2:_OB2 + 2] = b2.astype(np.float32)
    return w.reshape(1, _WLEN)


def _spill_extra_waits(nc):
    """This walrus build accepts at most ONE embedded sync-wait per
    instruction ("Too many sync wait commands"). Move any extra waits onto
    NoOp instructions inserted right before the owner on the same engine —
    the sequencer blocks at the NoOp, which is semantically identical."""
    cnt = 0
    for fn in nc.m.functions:
        for bb in fn.blocks:
            out = []
            for inst in bb.instructions:
                si = inst.sync_info
                if si is not None and si.on_wait and len(si.on_wait) > 1:
                    waits = list(si.on_wait)
                    for w in waits[:-1]:
                        cnt += 1
                        out.append(mybir.InstNoOp(
                            name=f"wspill-{cnt}",
                            sync_info=mybir.SyncInfo(on_wait=[w], on_update=[]),
                            engine=inst.engine,
                            bass_nofuse=True,
                        ))
                    si.on_wait = [waits[-1]]
                out.append(inst)
            bb.instructions[:] = out


# --------------------------------------------------------------------------- #
# Device program.
# --------------------------------------------------------------------------- #
def _build(pl):
    n_cores, G, L, NLOC = pl.n_cores, pl.G, pl.L, pl.NLOC
    nc = bass.Bass(num_devices=n_cores)

    xfeat = nc.declare_dram_parameter("xfeat", [2, P, L], F32, isOutput=False)
    xnode = nc.declare_dram_parameter("xnode", [2, P, G], F32, isOutput=False)
    gidx = nc.declare_dram_parameter("gidx", [P, L], I32, isOutput=False)
    npads = nc.declare_dram_parameter("npads", [P, G], F32, isOutput=False)
    wflat = nc.declare_dram_parameter("wflat", [1, _WLEN], F32, isOutput=False)
    alpha1 = nc.declare_dram_parameter("alpha1", [P, 4, L], F32, isOutput=True)
    alpha2 = nc.declare_dram_parameter("alpha2", [P, L], F32, isOutput=True)
    out2 = nc.declare_dram_parameter("out2", [P, G, 2], F32, isOutput=True)

    t_loc = nc.dram_tensor("t_loc", [NLOC, 4], F32)
    t_glob = nc.dram_tensor("t_glob", [n_cores * NLOC, 4], F32, addr_space="Shared")

    with tile.TileContext(nc) as tc:
        with (
            tc.tile_pool(name="const", bufs=1) as cpool,
            tc.tile_pool(name="node", bufs=1) as npool,
            tc.tile_pool(name="big", bufs=1) as bpool,
            tc.tile_pool(name="work", bufs=2) as wpool,
            tc.tile_pool(name="dma", bufs=3) as dpool,
        ):
            # xs is only needed during the layer-1 edge stream; scope it in a
            # pool that closes before ee2 allocates so the two reuse the same
            # SBUF. (The gather-destination pool `dma` stays in fresh space:
            # DMA copies accept a single sync wait, so their first writes
            # cannot afford space-reuse hazards.)
            _xsp_cm = tc.tile_pool(name="xsp", bufs=1)
            xsp = _xsp_cm.__enter__()
            # ---------------- weights prep ---------------- #
            wb = cpool.tile([P, _WLEN], F32)
            nc.sync.dma_start(out=wb[:], in_=wflat[:].to_broadcast((P, _WLEN)))

            def w_ap(o, n=1):
                return wb[:, o:o + n]

            # v_s[k,h] = sum_c W1[k, 8h+c] * att_src1[h,c]  (and v_d)
            vs = cpool.tile([P, 8], F32)
            vd = cpool.tile([P, 8], F32)
            for out_t, att_o in ((vs, _OAS1), (vd, _OAD1)):
                prod = wpool.tile([P, 64], F32, tag="wprod")
                in0 = wb[:, _OW1:_OW1 + 64].rearrange(
                    "p (k h c) -> p k h c", k=2, h=4, c=8)
                in1 = wb[:, att_o:att_o + 32].rearrange(
                    "p (h c) -> p h c", h=4, c=8)[:, None, :, :].to_broadcast(
                    (P, 2, 4, 8))
                nc.vector.tensor_tensor(
                    out=prod[:].rearrange("p (k h c) -> p k h c", k=2, h=4, c=8),
                    in0=in0, in1=in1, op=OP.mult)
                nc.vector.tensor_reduce(
                    out=out_t[:],
                    in_=prod[:].rearrange("p (k h c) -> p k h c", k=2, h=4, c=8),
                    axis=AX.X, op=OP.add)

            # wts[c] = sum_j W2[c,j]*att_src2[j]; wtd likewise with att_dst2
            wts = cpool.tile([P, 32], F32)
            wtd = cpool.tile([P, 32], F32)
            w2_as_cj = wb[:, _OW2:_OW2 + 64].rearrange("p (c j) -> p c j", c=32, j=2)
            for out_t, att_o in ((wts, _OAS2), (wtd, _OAD2)):
                tmp = wpool.tile([P, 32], F32, tag="wtmp")
                nc.vector.tensor_scalar(
                    out=tmp[:], in0=w2_as_cj[:, :, 0], scalar1=w_ap(att_o),
                    scalar2=None, op0=OP.mult)
                nc.vector.scalar_tensor_tensor(
                    out=out_t[:], in0=w2_as_cj[:, :, 1], scalar=w_ap(att_o + 1),
                    in1=tmp[:], op0=OP.mult, op1=OP.add)

            # ub1 = elu(b1); asdum = as2-chain over ub1 (bitwise-matching chain)
            ub1 = cpool.tile([P, 32], F32)
            r32 = wpool.tile([P, 32], F32, tag="w32a")
            m32 = wpool.tile([P, 32], F32, tag="w32b")
            e32 = wpool.tile([P, 32], F32, tag="w32c")
            nc.scalar.activation(out=r32[:], in_=wb[:, _OB1:_OB1 + 32], func=ACT.Relu)
            nc.vector.tensor_scalar(
                out=m32[:], in0=wb[:, _OB1:_OB1 + 32], scalar1=0.0, scalar2=None,
                op0=OP.min)
            nc.scalar.activation(out=e32[:], in_=m32[:], func=ACT.Exp)
            nc.vector.scalar_tensor_tensor(
                out=ub1[:], in0=r32[:], scalar=-1.0, in1=e32[:],
                op0=OP.add, op1=OP.add)

            asdum = cpool.tile([P, 1], F32)
            acc_a = wpool.tile([P, 1], F32, tag="wacc_a")
            acc_b = wpool.tile([P, 1], F32, tag="wacc_b")
            accs = [acc_a, acc_b]
            nc.vector.tensor_scalar(
                out=accs[0][:], in0=ub1[:, 0:1], scalar1=wts[:, 0:1], scalar2=None,
                op0=OP.mult)
            for c in range(1, 32):
                dst_t = asdum if c == 31 else accs[c % 2]
                nc.vector.scalar_tensor_tensor(
                    out=dst_t[:], in0=ub1[:, c:c + 1], scalar=wts[:, c:c + 1],
                    in1=accs[(c - 1) % 2][:], op0=OP.mult, op1=OP.add)

            # ---------------- layer-1 edge stream ---------------- #
            ee = bpool.tile([P, 4, L], F32)
            gixall = bpool.tile([P, L], I32)
            nc.gpsimd.dma_start(out=gixall[:], in_=gidx[:])
            # tiny Pool-engine read of gixall: makes the Pool sequencer observe
            # the DMASW completion sem early, so the indirect gathers below
            # need no extra wait for it (DMA copies accept only 1 sync wait).
            obs = cpool.tile([1, 1], I32)
            nc.gpsimd.tensor_copy(out=obs[:], in_=gixall[0:1, 0:1])
            xs = xsp.tile([P, 2, L], F32)
            nc.sync.dma_start(
                out=xs[:], in_=xfeat[:].rearrange("f p l -> p f l"))
            s4 = npool.tile([P, 4, G], F32)
            xw0 = npool.tile([P, 4, G], F32)
            xw1 = npool.tile([P, 4, G], F32)
            npb = npool.tile([P, G], F32)
            nc.sync.dma_start(out=npb[:], in_=npads[:])
            xnb = npool.tile([P, 2, G], F32)
            nc.sync.dma_start(out=xnb[:], in_=xnode[:].rearrange("f p g -> p f g"))

            # a_d1 per dst node: adn1[p,h,g] = xnode@v_d
            adn1 = npool.tile([P, 4, G], F32)
            for h in range(4):
                tn = wpool.tile([P, G], F32, tag="tn")
                nc.vector.tensor_scalar(
                    out=tn[:], in0=xnb[:, 0, :], scalar1=vd[:, h:h + 1],
                    scalar2=None, op0=OP.mult)
                nc.vector.scalar_tensor_tensor(
                    out=adn1[:, h, :], in0=xnb[:, 1, :], scalar=vd[:, 4 + h:5 + h],
                    in1=tn[:], op0=OP.mult, op1=OP.add)
            # epad1 = exp(leaky(adn1)) — what a padded slot's ee evaluates to
            epad1 = npool.tile([P, 4, G], F32)
            lr1 = wpool.tile([P, 4, G], F32, tag="lr1")
            nc.vector.scalar_tensor_tensor(
                out=lr1[:], in0=adn1[:], scalar=NEG_SLOPE, in1=adn1[:],
                op0=OP.mult, op1=OP.max)
            nc.scalar.activation(out=epad1[:], in_=lr1[:], func=ACT.Exp)

            for (g0, ng, D, c0) in pl.sginfo:
                W = ng * D
                for h in range(4):
                    t0 = wpool.tile([P, W], F32, tag="t0")
                    t1 = wpool.tile([P, W], F32, tag="t1")
                    prh = wpool.tile([P, W], F32, tag="prh")
                    nc.vector.tensor_scalar(
                        out=t0[:], in0=xs[:, 0, c0:c0 + W], scalar1=vs[:, h:h + 1],
                        scalar2=None, op0=OP.mult)
                    nc.vector.scalar_tensor_tensor(
                        out=t1[:], in0=xs[:, 1, c0:c0 + W], scalar=vs[:, 4 + h:5 + h],
                        in1=t0[:], op0=OP.mult, op1=OP.add)
                    nc.vector.tensor_tensor(
                        out=t0[:].rearrange("p (g d) -> p g d", g=ng, d=D),
                        in0=t1[:].rearrange("p (g d) -> p g d", g=ng, d=D),
                        in1=adn1[:, h, g0:g0 + ng, None].to_broadcast((P, ng, D)),
                        op=OP.add)
                    # leaky relu: max(z, 0.2*z)
                    nc.vector.scalar_tensor_tensor(
                        out=prh[:], in0=t0[:], scalar=NEG_SLOPE, in1=t0[:],
                        op0=OP.mult, op1=OP.max)
                    nc.scalar.activation(
                        out=ee[:, h, c0:c0 + W], in_=prh[:], func=ACT.Exp)
                for h in range(4):
                    ee_g = ee[:, h, c0:c0 + W].rearrange("p (g d) -> p g d", g=ng, d=D)
                    nc.vector.tensor_reduce(
                        out=s4[:, h, g0:g0 + ng], in_=ee_g, axis=AX.X, op=OP.add)
                    for k, xw in ((0, xw0), (1, xw1)):
                        pr = wpool.tile([P, W], F32, tag="pr")
                        nc.vector.tensor_tensor(
                            out=pr[:], in0=ee[:, h, c0:c0 + W],
                            in1=xs[:, k, c0:c0 + W], op=OP.mult)
                        nc.vector.tensor_reduce(
                            out=xw[:, h, g0:g0 + ng],
                            in_=pr[:].rearrange("p (g d) -> p g d", g=ng, d=D),
                            axis=AX.X, op=OP.add)

            # ---------------- layer-1 node finalize ---------------- #
            _xsp_cm.__exit__(None, None, None)
            _e2_cm = tc.tile_pool(name="e2p", bufs=1)
            e2p = _e2_cm.__enter__()
            sinv1 = npool.tile([P, 4, G], F32)
            seps = wpool.tile([P, 4, G], F32, tag="seps")
            nc.vector.tensor_tensor(
                out=seps[:], in0=npb[:, None, :].to_broadcast((P, 4, G)),
                in1=epad1[:], op=OP.mult)
            nc.vector.tensor_tensor(
                out=seps[:], in0=s4[:], in1=seps[:], op=OP.subtract)
            nc.vector.tensor_scalar(
                out=seps[:], in0=seps[:], scalar1=1e-16, scalar2=None, op0=OP.add)
            nc.vector.reciprocal(out=sinv1[:], in_=seps[:])

            u = npool.tile([P, 32, G], F32)
            for hc in range(32):
                h = hc // 8
                ta = wpool.tile([P, G], F32, tag="ta")
                tb = wpool.tile([P, G], F32, tag="tb")
                nc.vector.tensor_scalar(
                    out=ta[:], in0=xw1[:, h, :], scalar1=w_ap(_OW1 + 32 + hc),
                    scalar2=None, op0=OP.mult)
                nc.vector.scalar_tensor_tensor(
                    out=tb[:], in0=xw0[:, h, :], scalar=w_ap(_OW1 + hc), in1=ta[:],
                    op0=OP.mult, op1=OP.add)
                nc.vector.tensor_tensor(
                    out=ta[:], in0=tb[:], in1=sinv1[:, h, :], op=OP.mult)
                rt = wpool.tile([P, G], F32, tag="rt")
                mt = wpool.tile([P, G], F32, tag="mt")
                et = wpool.tile([P, G], F32, tag="et")
                nc.scalar.activation(
                    out=rt[:], in_=ta[:], func=ACT.Relu, bias=w_ap(_OB1 + hc))
                nc.vector.tensor_scalar(
                    out=mt[:], in0=ta[:], scalar1=w_ap(_OB1 + hc), scalar2=0.0,
                    op0=OP.add, op1=OP.min)
                nc.scalar.activation(out=et[:], in_=mt[:], func=ACT.Exp)
                nc.vector.scalar_tensor_tensor(
                    out=u[:, hc, :], in0=rt[:], scalar=-1.0, in1=et[:],
                    op0=OP.add, op1=OP.add)

            # t-pack: [t0, t1, a_s2, 0] per node; a_d2 separately
            tpk = npool.tile([P, G, 4], F32)
            ad2n = npool.tile([P, G], F32)
            nc.vector.memset(tpk[:], 0.0)
            chains = (
                (lambda c: w_ap(_OW2 + 2 * c), tpk[:, :, 0]),
                (lambda c: w_ap(_OW2 + 2 * c + 1), tpk[:, :, 1]),
                (lambda c: wts[:, c:c + 1], tpk[:, :, 2]),
                (lambda c: wtd[:, c:c + 1], ad2n[:]),
            )
            for (scl, outslice) in chains:
                ca = wpool.tile([P, G], F32, tag="ca")
                cb = wpool.tile([P, G], F32, tag="cb")
                cc = [ca, cb]
                nc.vector.tensor_scalar(
                    out=cc[0][:], in0=u[:, 0, :], scalar1=scl(0), scalar2=None,
                    op0=OP.mult)
                for c in range(1, 32):
                    dst_t = outslice if c == 31 else cc[c % 2][:]
                    nc.vector.scalar_tensor_tensor(
                        out=dst_t, in0=u[:, c, :], scalar=scl(c),
                        in1=cc[(c - 1) % 2][:], op0=OP.mult, op1=OP.add)

            # epad = exp(leaky(a_d2 + asdum))
            epad = npool.tile([P, G], F32)
            pp = wpool.tile([P, G], F32, tag="pp")
            nc.vector.tensor_scalar(
                out=pp[:], in0=ad2n[:], scalar1=asdum[:, 0:1], scalar2=None,
                op0=OP.add)
            nc.vector.scalar_tensor_tensor(
                out=pp[:], in0=pp[:], scalar=NEG_SLOPE, in1=pp[:],
                op0=OP.mult, op1=OP.max)
            nc.scalar.activation(out=epad[:], in_=pp[:], func=ACT.Exp)

            # publish t-table, allgather
            nc.sync.dma_start(
                out=t_loc[:].rearrange("(g p) f -> p g f", p=P, g=G), in_=tpk[:])
            nc.gpsimd.collective_compute(
                "AllGather", OP.bypass,
                replica_groups=[list(range(pl.n_cores))],
                ins=[t_loc[:]], outs=[t_glob[:]])

            # ---------------- alpha1 ---------------- #
            for (g0, ng, D, c0) in pl.sginfo:
                W = ng * D
                for h in range(4):
                    a1 = wpool.tile([P, W], F32, tag="a1")
                    nc.vector.tensor_tensor(
                        out=a1[:].rearrange("p (g d) -> p g d", g=ng, d=D),
                        in0=ee[:, h, c0:c0 + W].rearrange(
                            "p (g d) -> p g d", g=ng, d=D),
                        in1=sinv1[:, h, g0:g0 + ng, None].to_broadcast((P, ng, D)),
                        op=OP.mult)
                    nc.sync.dma_start(out=alpha1[:, h, c0:c0 + W], in_=a1[:])

            # ---------------- layer-2 edge stream ---------------- #
            ee2 = e2p.tile([P, L], F32)
            s2 = npool.tile([P, G], F32)
            xw20 = npool.tile([P, G], F32)
            xw21 = npool.tile([P, G], F32)
            for (g0, ng, D, c0) in pl.sginfo:
                W = ng * D
                gt = dpool.tile([P, W, 4], F32, tag="gt")
                # the vector-indirect DMA consumes exactly one offset per
                # partition per instruction; gather column by column
                for j in range(W):
                    nc.gpsimd.indirect_dma_start(
                        out=gt[:, j, :], out_offset=None, in_=t_glob[:],
                        in_offset=IndirectOffsetOnAxis(
                            ap=gixall[:, c0 + j:c0 + j + 1], axis=0))
                p2 = wpool.tile([P, W], F32, tag="p2")
                nc.vector.tensor_tensor(
                    out=p2[:].rearrange("p (g d) -> p g d", g=ng, d=D),
                    in0=gt[:, :, 2].rearrange("p (g d) -> p g d", g=ng, d=D),
                    in1=ad2n[:, g0:g0 + ng, None].to_broadcast((P, ng, D)),
                    op=OP.add)
                nc.vector.scalar_tensor_tensor(
                    out=p2[:], in0=p2[:], scalar=NEG_SLOPE, in1=p2[:],
                    op0=OP.mult, op1=OP.max)
                nc.scalar.activation(out=ee2[:, c0:c0 + W], in_=p2[:], func=ACT.Exp)
                nc.vector.tensor_reduce(
                    out=s2[:, g0:g0 + ng],
                    in_=ee2[:, c0:c0 + W].rearrange("p (g d) -> p g d", g=ng, d=D),
                    axis=AX.X, op=OP.add)
                for k, xw in ((0, xw20), (1, xw21)):
                    pr2 = wpool.tile([P, W], F32, tag="pr2")
                    nc.vector.tensor_tensor(
                        out=pr2[:], in0=ee2[:, c0:c0 + W], in1=gt[:, :, k],
                        op=OP.mult)
                    nc.vector.tensor_reduce(
                        out=xw[:, g0:g0 + ng],
                        in_=pr2[:].rearrange("p (g d) -> p g d", g=ng, d=D),
                        axis=AX.X, op=OP.add)

            # ---------------- layer-2 finalize ---------------- #
            sinv2 = npool.tile([P, G], F32)
            t2a = wpool.tile([P, G], F32, tag="t2a")
            nc.vector.tensor_tensor(out=t2a[:], in0=npb[:], in1=epad[:], op=OP.mult)
            nc.vector.tensor_tensor(out=t2a[:], in0=s2[:], in1=t2a[:], op=OP.subtract)
            nc.vector.tensor_scalar(
                out=t2a[:], in0=t2a[:], scalar1=1e-16, scalar2=None, op0=OP.add)
            nc.vector.reciprocal(out=sinv2[:], in_=t2a[:])

            o2 = npool.tile([P, G, 2], F32)
            for j, xw in ((0, xw20), (1, xw21)):
                t2b = wpool.tile([P, G], F32, tag="t2b")
                nc.vector.tensor_tensor(out=t2b[:], in0=xw[:], in1=sinv2[:], op=OP.mult)
                nc.vector.tensor_scalar(
                    out=o2[:, :, j], in0=t2b[:], scalar1=w_ap(_OB2 + j),
                    scalar2=None, op0=OP.add)
            nc.sync.dma_start(out=out2[:], in_=o2[:])

            for (g0, ng, D, c0) in pl.sginfo:
                W = ng * D
                a2 = wpool.tile([P, W], F32, tag="a2")
                nc.vector.tensor_tensor(
                    out=a2[:].rearrange("p (g d) -> p g d", g=ng, d=D),
                    in0=ee2[:, c0:c0 + W].rearrange("p (g d) -> p g d", g=ng, d=D),
                    in1=sinv2[:, g0:g0 + ng, None].to_broadcast((P, ng, D)),
                    op=OP.mult)
                nc.sync.dma_start(out=alpha2[:, c0:c0 + W], in_=a2[:])

            _e2_cm.__exit__(None, None, None)

    _spill_extra_waits(nc)
    return nc


# --------------------------------------------------------------------------- #
# Entry points.
# --------------------------------------------------------------------------- #
def gat_forward(inputs, n_cores=8, trace=False, sim=False, tmpdir=None):
    x = np.asarray(inputs["x"], dtype=np.float32)
    edge_index = np.asarray(inputs["edge_index"])
    pl = _prepare(x, edge_index, n_cores)
    wfl = _pack_weights(
        np.asarray(inputs["W1"]), np.asarray(inputs["att_src1"]),
        np.asarray(inputs["att_dst1"]), np.asarray(inputs["b1"]),
        np.asarray(inputs["W2"]), np.asarray(inputs["att_src2"]),
        np.asarray(inputs["att_dst2"]), np.asarray(inputs["b2"]))
    nc = _build(pl)

    in_maps = [
        {"xfeat": pl.xfeat[c], "xnode": pl.xnode[c], "gidx": pl.gidx[c],
         "npads": pl.npads[c], "wflat": wfl}
        for c in range(n_cores)
    ]
    if sim:
        from concourse.bass_interp import MultiCoreSim
        ms = MultiCoreSim(nc, n_cores, num_workers=min(8, n_cores))
        for c in range(n_cores):
            for k, v in in_maps[c].items():
                ms.cores[c].tensor(k)[:] = v
        ms.simulate()
        results = [
            {k: np.array(ms.cores[c].tensor(k))
             for k in ("alpha1", "alpha2", "out2")}
            for c in range(n_cores)
        ]
        exec_ns = None
    else:
        r = run_bass_kernel_spmd(
            nc, in_maps, list(range(n_cores)), trace=trace, tmpdir=tmpdir)
        results = r.results
        exec_ns = r.exec_time_ns

    # unshard
    E, L, Pdim = pl.E, pl.L, P
    a1 = np.stack([results[c]["alpha1"] for c in range(n_cores)])  # [C,128,4,L]
    a2 = np.stack([results[c]["alpha2"] for c in range(n_cores)])  # [C,128,L]
    o2 = np.stack([results[c]["out2"] for c in range(n_cores)])    # [C,128,G,2]

    ce = pl.core_of_edge
    prow = pl.edge_slot // L
    pcol = pl.edge_slot % L
    alpha1_full = a1[ce, prow, :, pcol].astype(np.float32)         # [E,4]
    alpha2_full = a2[ce, prow, pcol][:, None].astype(np.float32)   # [E,1]
    rk = pl.rank_of
    cn = np.arange(pl.N) // pl.NSH
    out_full = o2[cn, rk % P, rk // P, :].astype(np.float32)       # [N,2]
    return (out_full, alpha1_full, alpha2_full), exec_ns


def kernel(**inputs):
    (out, a1, a2), _ = gat_forward(inputs, n_cores=8)
    return out, a1, a2
